# revision 21
# baseline (speedup 1.0000x reference)
"""Self-contained Bass/Trainium2 kernel for nn_Attention (B=4, N=2048, D=1024, H=16, dh=64).

Sharding: 8 cores = (batch b in 0..3) x (sequence half in 0..1).
Each core computes attention output for its 1024 rows; full-sequence K/V are
computed on-core. Host feeds x[b] transposed (fp16) with the core's own rows
last; softmax is order-invariant in j.

Pipeline: a single software-pipelined stream. Per head pair: dots (row-packed
64-contraction matmul pairs) -> 1536-wide exp on ScalarE -> AV with the
attention tile as the stationary operand (output [i, d] in PSUM, accumulated
over j-tiles, with a ones-column in v providing softmax denominators).
K/Q/V projections for later pairs are interleaved into the PE slack under the
ACT-bound exp stream so the tensor engine never idles long enough to be
re-throttled by HAM. Normalization is a per-partition reciprocal +
tensor_scalar. Out-projection runs at the end from pair-transposed context.
"""

import sys
import numpy as np

sys.path.insert(0, "/opt/trn_rl_repo")

B, N, DIM = 4, 2048, 1024
HEADS, DH = 16, 64
SCALE = DH ** -0.5  # 0.125
NC = 8
HALF = N // 2            # rows per core
PAIRS = HEADS // 2       # 8 head pairs
CT = DIM // 128          # 8 channel tiles
JT = N // 128            # 16 j tiles
ICH = 256                # i-chunk width for po accumulation
NCH = HALF // ICH        # 4 chunks
SLOT = 256               # ring slot width (== ICH)
GW = 6                   # slots per exp group (1536 wide)
VROW = DH + 1            # 65: per-head v width incl. ones column
SPP = NCH * JT * 2       # slots per pair = 128
NSLOT = PAIRS * SPP      # 1024
NGRP = (NSLOT + GW - 1) // GW

_compiled = None


def _build():
    from collections import deque

    import concourse.tile as tile
    from concourse import bacc, mybir
    from concourse.masks import make_identity

    f32 = mybir.dt.float32
    f32r = mybir.dt.float32r
    f16 = mybir.dt.float16
    EXP = mybir.ActivationFunctionType.Exp
    MULT = mybir.AluOpType.mult

    nc = bacc.Bacc("TRN2", target_bir_lowering=False, debug=False, num_devices=NC)

    XT = nc.dram_tensor("x", (DIM, N), f16, kind="ExternalInput").ap()
    WV = nc.dram_tensor("wv", (DIM, DIM), f16, kind="ExternalInput").ap()
    WK = nc.dram_tensor("wk", (DIM, DIM), f16, kind="ExternalInput").ap()
    WQ = nc.dram_tensor("wq", (DIM, DIM), f16, kind="ExternalInput").ap()
    WO = nc.dram_tensor("wo", (DIM, DIM), f16, kind="ExternalInput").ap()
    BOUT = nc.dram_tensor("b_out", (DIM,), f32, kind="ExternalInput").ap()
    Y = nc.dram_tensor("y", (HALF, DIM), f32, kind="ExternalOutput").ap()

    def decode(s):
        # slot -> (pair, cq, jt, h, ihalf); a 512-wide i-chunk (pair, cq) spans
        # 64 slots ordered (jt, h, ihalf); dots MMs are 512 wide (2 slots).
        pair, u = divmod(s, SPP)
        cq, v = divmod(u, 64)
        jt, r3 = divmod(v, 4)
        h, ihalf = divmod(r3, 2)
        return pair, cq, jt, h, ihalf

    with tile.TileContext(nc) as tc:
        with tc.tile_pool(name="persist", bufs=1) as persist:
            ident32 = persist.tile([128, 128], f32, tag="id32")
            make_identity(nc, ident32[:])
            ident = persist.tile([128, 128], f32r, tag="ident")
            nc.vector.tensor_copy(ident[:], ident32[:])

            bias_src = persist.tile([1, DIM], f32, tag="bsrc")
            nc.sync.dma_start(bias_src[:], BOUT.rearrange("(o d) -> o d", o=1))
            bias = persist.tile([128, DIM], f32, tag="bias")
            nc.gpsimd.partition_broadcast(bias[:], bias_src[0:1, :])

            xT = [persist.tile([128, N], f16, tag="xT", bufs=CT, name=f"xT{c}")
                  for c in range(CT)]
            for c in range(CT):
                nc.sync.dma_start(xT[c][:], XT[c * 128:(c + 1) * 128, :])
            wv_t = [persist.tile([128, DIM], f16, tag="wv", bufs=CT, name=f"wv{c}")
                    for c in range(CT)]
            for c in range(CT):
                nc.sync.dma_start(wv_t[c][:], WV[c * 128:(c + 1) * 128, :])
            wout_t = [persist.tile([128, DIM], f16, tag="wo", bufs=CT, name=f"wo{c}")
                      for c in range(CT)]
            for c in range(CT):
                nc.sync.dma_start(wout_t[c][:], WO[c * 128:(c + 1) * 128, :])

            # v in natural [j, head-major (64 dims + ones)] layout
            v_nat = persist.tile([128, JT * HEADS * VROW], f16, tag="vnat")
            ones_view = v_nat.rearrange("p (x c) -> p x c", c=VROW)[:, :, DH:VROW]
            nc.gpsimd.memset(ones_view, 1.0)

            # normalized attention output, [i(128), pair-dims(128)] per (pair, itg)
            ctx_t = [[persist.tile([128, 128], f32r, tag="ctxi", bufs=64,
                                   name=f"ctx{p}_{i}") for i in range(8)]
                     for p in range(PAIRS)]

            with tc.tile_pool(name="kqpool", bufs=1) as kqp, \
                 tc.tile_pool(name="wkqpool", bufs=1) as wkqp, \
                 tc.tile_pool(name="ring", bufs=1) as ringp, \
                 tc.tile_pool(name="small", bufs=1) as smallp, \
                 tc.tile_pool(name="ppp", bufs=1, space="PSUM") as ppp, \
                 tc.tile_pool(name="pop", bufs=1, space="PSUM") as pop, \
                 tc.tile_pool(name="projp", bufs=1, space="PSUM") as projp:

                kT, qT, wkt, wqt = {}, {}, {}, {}
                pp_tiles, at_tiles, po_tiles = {}, {}, {}

                def emit_wdma(p):
                    wk = wkqp.tile([128, DIM], f16, tag="wk", bufs=2, name=f"wk{p}")
                    nc.sync.dma_start(wk.rearrange("p (t d) -> p t d", d=128),
                                      WK[:, p * 128:(p + 1) * 128]
                                      .rearrange("(t p) d -> p t d", p=128))
                    wq = wkqp.tile([128, DIM], f16, tag="wq", bufs=2, name=f"wq{p}")
                    nc.sync.dma_start(wq.rearrange("p (t d) -> p t d", d=128),
                                      WQ[:, p * 128:(p + 1) * 128]
                                      .rearrange("(t p) d -> p t d", p=128))
                    wkt[p], wqt[p] = wk, wq

                def emit_kq_chunk(p, idx):
                    if idx == 0:
                        kT[p] = kqp.tile([128, N], f32r, tag="kT", bufs=2,
                                         name=f"kT{p}")
                        qT[p] = kqp.tile([128, HALF], f32r, tag="qT", bufs=2,
                                         name=f"qT{p}")
                    ps = projp.tile([128, 512], f32, tag="proj", bufs=1,
                                    name=f"prj{p}_{idx}")
                    if idx < 4:
                        for ct in range(CT):
                            nc.tensor.matmul(
                                ps[:], wkt[p][:, ct * 128:(ct + 1) * 128],
                                xT[ct][:, idx * 512:(idx + 1) * 512],
                                start=(ct == 0), stop=(ct == CT - 1))
                        nc.vector.tensor_copy(kT[p][:, idx * 512:(idx + 1) * 512],
                                              ps[:])
                    else:
                        q = idx - 4
                        for ct in range(CT):
                            nc.tensor.matmul(
                                ps[:], wqt[p][:, ct * 128:(ct + 1) * 128],
                                xT[ct][:, HALF + q * 512:HALF + (q + 1) * 512],
                                start=(ct == 0), stop=(ct == CT - 1))
                        nc.vector.tensor_copy(qT[p][:, q * 512:(q + 1) * 512],
                                              ps[:])

                def emit_v_bundle(p, b):
                    ps = projp.tile([128, 512], f32, tag="proj", bufs=1,
                                    name=f"vb{p}_{b}")
                    for jq in range(4):
                        jt = 4 * b + jq
                        for ct in range(CT):
                            nc.tensor.matmul(
                                ps[:, jq * 128:(jq + 1) * 128],
                                xT[ct][:, jt * 128:(jt + 1) * 128],
                                wv_t[ct][:, p * 128:(p + 1) * 128],
                                start=(ct == 0), stop=(ct == CT - 1))
                    src = ps.rearrange("p (j h c) -> p j h c", j=4, h=2)
                    dst = v_nat.rearrange("p (j hh c) -> p j hh c", hh=HEADS,
                                          c=VROW)[:, 4 * b:4 * b + 4,
                                                  2 * p:2 * p + 2, 0:DH]
                    nc.vector.tensor_copy(dst, src)

                # slots are laid out in "decades" of 10: an even (1536-wide,
                # 6-slot) group then an odd (1024-wide, 4-slot) group, so the
                # pp double-buffer fits 5 PSUM banks (3 + 2).
                def slot_group(s):
                    d, r = divmod(s, 10)
                    if r < 6:
                        return 2 * d, r * SLOT
                    return 2 * d + 1, (r - 6) * SLOT

                def group_slots(g):
                    d, par = divmod(g, 2)
                    base = d * 10 + (6 if par else 0)
                    n = 4 if par else 6
                    return range(base, min(NSLOT, base + n))

                def emit_dots(s):
                    # one f32r matmul covers slots (s, s+1): 512-wide moving
                    # (f32r matmuls with a 256-wide moving operand hang the HW)
                    pair, cq, jt, h, ihalf = decode(s)
                    if ihalf != 0:
                        return
                    g, col = slot_group(s)
                    if g not in pp_tiles:
                        if g % 2 == 0:
                            pp_tiles[g] = ppp.tile([128, 6 * SLOT], f32,
                                                   tag="ppA", bufs=1,
                                                   name=f"pp{g}")
                        else:
                            pp_tiles[g] = ppp.tile([128, 4 * SLOT], f32,
                                                   tag="ppB", bufs=1,
                                                   name=f"pp{g}")
                    nc.tensor.matmul(
                        pp_tiles[g][:, col:col + 512],
                        kT[pair][h * 64:(h + 1) * 64, jt * 128:(jt + 1) * 128],
                        qT[pair][h * 64:(h + 1) * 64, cq * 512:(cq + 1) * 512],
                        start=True, stop=True)

                def emit_exp(g):
                    n = len(group_slots(g))
                    w = n * SLOT
                    if g % 2 == 0:
                        at = ringp.tile([128, 6 * SLOT], f16, tag="atA", bufs=7,
                                        name=f"at{g}")
                    else:
                        at = ringp.tile([128, 4 * SLOT], f16, tag="atB", bufs=7,
                                        name=f"at{g}")
                    nc.scalar.activation(at[:, 0:w], pp_tiles[g][:, 0:w], EXP,
                                         bias=0.0, scale=SCALE)
                    at_tiles[g] = at
                    del pp_tiles[g]

                def slot_of(pair, cq, jt, h, ihalf):
                    return ((pair * 2 + cq) * 16 + jt) * 4 + h * 2 + ihalf

                def av_mm(s, it):
                    # one AV matmul: at-slice stationary, v (+ones col) moving
                    pair, cq, jt, h, ihalf = decode(s)
                    g, col = slot_group(s)
                    po = po_tiles[(pair, cq)][h]
                    hg = 2 * pair + h
                    nc.tensor.matmul(
                        po[:, it * VROW:(it + 1) * VROW],
                        at_tiles[g][:, col + (it % 2) * 128:
                                    col + (it % 2 + 1) * 128],
                        v_nat[:, jt * HEADS * VROW + hg * VROW:
                              jt * HEADS * VROW + (hg + 1) * VROW],
                        start=(jt == 0), stop=(jt == JT - 1))

                def emit_norm(pair, cq):
                    poh = po_tiles.pop((pair, cq))
                    for h in range(2):
                        for it in range(4):
                            off = it * VROW
                            rs = smallp.tile([128, 1], f32, tag="rs", bufs=8,
                                             name=f"rs{pair}_{cq}_{h}_{it}")
                            nc.vector.reciprocal(rs[:],
                                                 poh[h][:, off + DH:off + DH + 1])
                            itg = cq * 4 + it
                            nc.vector.tensor_scalar(
                                ctx_t[pair][itg][:, h * 64:(h + 1) * 64],
                                poh[h][:, off:off + DH], rs[:], None, op0=MULT)

                def emit_av(g):
                    # streamed it=0 chains for this group's ihalf=0 slots; when
                    # a (pair, cq) chunk's last slot is exp'd, burst the it=1..3
                    # chains (sequential per PSUM bank, interleaved across the
                    # two banks) and normalize.
                    for s in group_slots(g):
                        pair, cq, jt, h, ihalf = decode(s)
                        if (pair, cq) not in po_tiles:
                            poA = pop.tile([128, 512], f32, tag="poA", bufs=1,
                                           name=f"poA{pair}_{cq}")
                            poB = pop.tile([128, 512], f32, tag="poB", bufs=1,
                                           name=f"poB{pair}_{cq}")
                            po_tiles[(pair, cq)] = (poA, poB)
                        if ihalf == 0:
                            av_mm(s, 0)
                        if jt == JT - 1 and h == 1 and ihalf == 1:
                            for it in (1, 2, 3):
                                for jt2 in range(JT):
                                    for h2 in range(2):
                                        av_mm(slot_of(pair, cq, jt2, h2, it // 2),
                                              it)
                            emit_norm(pair, cq)

                # ---- schedule ----
                proj_q = deque()
                projected = set()

                def push_pair(p):
                    if p < PAIRS and p not in projected:
                        projected.add(p)
                        proj_q.append(("w", p))
                        for i in range(6):
                            proj_q.append(("kq", p, i))
                        for b in range(4):
                            proj_q.append(("v", p, b))

                def pop_bundle():
                    if not proj_q:
                        return
                    b = proj_q.popleft()
                    if b[0] == "w":
                        emit_wdma(b[1])
                    elif b[0] == "kq":
                        emit_kq_chunk(b[1], b[2])
                    else:
                        emit_v_bundle(b[1], b[2])

                push_pair(0)
                while proj_q:
                    pop_bundle()
                push_pair(1)

                last_exp = -1
                for s in range(NSLOT):
                    pair = s // SPP
                    if s % SPP == 0:
                        while proj_q and proj_q[0][1] <= pair:
                            pop_bundle()
                        push_pair(pair + 1)
                    emit_dots(s)
                    r = s % 10
                    if r == 5 or r == 9 or s == NSLOT - 1:
                        g, _ = slot_group(s)
                        emit_exp(g)
                        if g > 0:
                            emit_av(g - 1)
                        last_exp = g
                        pop_bundle()
                emit_av(last_exp)
                while proj_q:
                    pop_bundle()

            # ---- tail: transpose ctx to [d, i] and out-project ----
            with tc.tile_pool(name="tail", bufs=1) as tailp, \
                 tc.tile_pool(name="tpsum", bufs=1, space="PSUM") as tpsum:
                for itg in range(8):
                    ctxT = tailp.tile([128, PAIRS * 128], f16, tag="ctxT", bufs=2,
                                      name=f"ctxT{itg}")
                    for p in range(PAIRS):
                        tp = tpsum.tile([128, 128], f32r, tag="tp", bufs=4,
                                        name=f"tp{itg}_{p}")
                        nc.tensor.transpose(tp[:], ctx_t[p][itg][:], ident[:])
                        nc.vector.tensor_copy(ctxT[:, p * 128:(p + 1) * 128], tp[:])
                    for ep in range(2):
                        yp = tpsum.tile([128, 512], f32, tag="yp", bufs=2,
                                        name=f"yp{itg}_{ep}")
                        for p in range(PAIRS):
                            nc.tensor.matmul(
                                yp[:], ctxT[:, p * 128:(p + 1) * 128],
                                wout_t[p][:, ep * 512:(ep + 1) * 512],
                                start=(p == 0), stop=(p == PAIRS - 1))
                        ysb = tailp.tile([128, 512], f32, tag="ysb", bufs=3,
                                         name=f"ysb{itg}_{ep}")
                        nc.vector.tensor_add(ysb[:], yp[:],
                                             bias[:, ep * 512:(ep + 1) * 512])
                        nc.sync.dma_start(
                            Y[itg * 128:(itg + 1) * 128, ep * 512:(ep + 1) * 512],
                            ysb[:])

    nc.compile()
    return nc


def _get_compiled():
    global _compiled
    if _compiled is None:
        _compiled = _build()
    return _compiled


def make_in_maps(x, w_qkv, w_out, b_out):
    x = np.asarray(x, dtype=np.float32)
    w_qkv = np.asarray(w_qkv, dtype=np.float32)
    w_out = np.asarray(w_out, dtype=np.float32)
    b_out = np.asarray(b_out, dtype=np.float32)

    wq = np.ascontiguousarray(w_qkv[:, 0:DIM].astype(np.float16))
    wk = np.ascontiguousarray(w_qkv[:, DIM:2 * DIM].astype(np.float16))
    wv = np.ascontiguousarray(w_qkv[:, 2 * DIM:3 * DIM].astype(np.float16))
    wo = np.ascontiguousarray(w_out.astype(np.float16))

    in_maps = []
    for c in range(NC):
        b, half = divmod(c, 2)
        other = x[b][(1 - half) * HALF:(2 - half) * HALF]
        mine = x[b][half * HALF:(half + 1) * HALF]
        xb = np.ascontiguousarray(
            np.concatenate([other, mine], axis=0).T.astype(np.float16))
        in_maps.append({"x": xb, "wv": wv, "wk": wk, "wq": wq, "wo": wo,
                        "b_out": b_out})
    return in_maps


def kernel(x, w_qkv, w_out, b_out):
    from concourse.bass_utils import run_bass_kernel_spmd

    nc = _get_compiled()
    in_maps = make_in_maps(x, w_qkv, w_out, b_out)
    res = run_bass_kernel_spmd(nc, in_maps, core_ids=list(range(NC)))

    out = np.empty((B, N, DIM), dtype=np.float32)
    for c in range(NC):
        b, half = divmod(c, 2)
        out[b, half * HALF:(half + 1) * HALF] = res.results[c]["y"]
    return out


# revision 22
# speedup vs baseline: 1.1093x; 1.1093x over previous
"""Self-contained Bass/Trainium2 kernel for nn_Attention (B=4, N=2048, D=1024, H=16, dh=64).

Sharding: 8 cores = (batch b in 0..3) x (sequence half in 0..1).
Each core computes attention output for its 1024 rows; full-sequence K/V are
computed on-core. Host feeds x[b] transposed (fp16) with the core's own rows
last; softmax is order-invariant in j.

Pipeline: a single software-pipelined stream. Per head pair: dots (row-packed
64-contraction matmul pairs) -> 1536-wide exp on ScalarE -> AV with the
attention tile as the stationary operand (output [i, d] in PSUM, accumulated
over j-tiles, with a ones-column in v providing softmax denominators).
K/Q/V projections for later pairs are interleaved into the PE slack under the
ACT-bound exp stream so the tensor engine never idles long enough to be
re-throttled by HAM. Normalization is a per-partition reciprocal +
tensor_scalar. Out-projection runs at the end from pair-transposed context.
"""

import sys
import numpy as np

sys.path.insert(0, "/opt/trn_rl_repo")

B, N, DIM = 4, 2048, 1024
HEADS, DH = 16, 64
SCALE = DH ** -0.5  # 0.125
NC = 8
HALF = N // 2            # rows per core
PAIRS = HEADS // 2       # 8 head pairs
CT = DIM // 128          # 8 channel tiles
JT = N // 128            # 16 j tiles
ICH = 256                # i-chunk width for po accumulation
NCH = HALF // ICH        # 4 chunks
SLOT = 256               # ring slot width (== ICH)
GW = 6                   # slots per exp group (1536 wide)
VROW = DH + 1            # 65: per-head v width incl. ones column
SPP = NCH * JT * 2       # slots per pair = 128
NSLOT = PAIRS * SPP      # 1024
NGRP = (NSLOT + GW - 1) // GW

_compiled = None


def _build():
    from collections import deque

    import concourse.tile as tile
    from concourse import bacc, mybir
    from concourse.masks import make_identity

    f32 = mybir.dt.float32
    f32r = mybir.dt.float32r
    f16 = mybir.dt.float16
    EXP = mybir.ActivationFunctionType.Exp
    MULT = mybir.AluOpType.mult

    nc = bacc.Bacc("TRN2", target_bir_lowering=False, debug=False, num_devices=NC)

    XT = nc.dram_tensor("x", (DIM, N), f16, kind="ExternalInput").ap()
    WV = nc.dram_tensor("wv", (DIM, DIM), f16, kind="ExternalInput").ap()
    WK = nc.dram_tensor("wk", (DIM, DIM), f16, kind="ExternalInput").ap()
    WQ = nc.dram_tensor("wq", (DIM, DIM), f16, kind="ExternalInput").ap()
    WO = nc.dram_tensor("wo", (DIM, DIM), f16, kind="ExternalInput").ap()
    BOUT = nc.dram_tensor("b_out", (DIM,), f32, kind="ExternalInput").ap()
    Y = nc.dram_tensor("y", (HALF, DIM), f32, kind="ExternalOutput").ap()

    def decode(s):
        # slot -> (pair, cq, jt, h, ihalf); a 512-wide i-chunk (pair, cq) spans
        # 64 slots ordered (jt, h, ihalf); dots MMs are 512 wide (2 slots).
        pair, u = divmod(s, SPP)
        cq, v = divmod(u, 64)
        jt, r3 = divmod(v, 4)
        h, ihalf = divmod(r3, 2)
        return pair, cq, jt, h, ihalf

    with tile.TileContext(nc) as tc:
        with tc.tile_pool(name="persist", bufs=1) as persist:
            ident32 = persist.tile([128, 128], f32, tag="id32")
            make_identity(nc, ident32[:])
            ident = persist.tile([128, 128], f32r, tag="ident")
            nc.vector.tensor_copy(ident[:], ident32[:])

            bias_src = persist.tile([1, DIM], f32, tag="bsrc")
            nc.sync.dma_start(bias_src[:], BOUT.rearrange("(o d) -> o d", o=1))
            bias = persist.tile([128, DIM], f32, tag="bias")
            nc.gpsimd.partition_broadcast(bias[:], bias_src[0:1, :])

            xT = [persist.tile([128, N], f16, tag="xT", bufs=CT, name=f"xT{c}")
                  for c in range(CT)]
            for c in range(CT):
                nc.sync.dma_start(xT[c][:], XT[c * 128:(c + 1) * 128, :])
            wv_t = [persist.tile([128, DIM], f16, tag="wv", bufs=CT, name=f"wv{c}")
                    for c in range(CT)]
            for c in range(CT):
                nc.sync.dma_start(wv_t[c][:], WV[c * 128:(c + 1) * 128, :])
            wout_t = [persist.tile([128, DIM], f16, tag="wo", bufs=CT, name=f"wo{c}")
                      for c in range(CT)]
            for c in range(CT):
                nc.sync.dma_start(wout_t[c][:], WO[c * 128:(c + 1) * 128, :])

            # v in natural [j, head-major (64 dims + ones)] layout
            v_nat = persist.tile([128, JT * HEADS * VROW], f16, tag="vnat")
            ones_view = v_nat.rearrange("p (x c) -> p x c", c=VROW)[:, :, DH:VROW]
            nc.gpsimd.memset(ones_view, 1.0)

            # normalized attention output, [i(128), pair-dims(128)] per (pair, itg)
            ctx_t = [[persist.tile([128, 128], f32r, tag="ctxi", bufs=64,
                                   name=f"ctx{p}_{i}") for i in range(8)]
                     for p in range(PAIRS)]

            with tc.tile_pool(name="kqpool", bufs=1) as kqp, \
                 tc.tile_pool(name="wkqpool", bufs=1) as wkqp, \
                 tc.tile_pool(name="ring", bufs=1) as ringp, \
                 tc.tile_pool(name="small", bufs=1) as smallp, \
                 tc.tile_pool(name="ppp", bufs=1, space="PSUM") as ppp, \
                 tc.tile_pool(name="pop", bufs=1, space="PSUM") as pop, \
                 tc.tile_pool(name="projp", bufs=1, space="PSUM") as projp:

                kT, qT, wkt, wqt = {}, {}, {}, {}
                pp_tiles, at_tiles, po_tiles = {}, {}, {}

                def emit_wdma(p):
                    wk = wkqp.tile([128, DIM], f16, tag="wk", bufs=2, name=f"wk{p}")
                    nc.sync.dma_start(wk.rearrange("p (t d) -> p t d", d=128),
                                      WK[:, p * 128:(p + 1) * 128]
                                      .rearrange("(t p) d -> p t d", p=128))
                    wq = wkqp.tile([128, DIM], f16, tag="wq", bufs=2, name=f"wq{p}")
                    nc.sync.dma_start(wq.rearrange("p (t d) -> p t d", d=128),
                                      WQ[:, p * 128:(p + 1) * 128]
                                      .rearrange("(t p) d -> p t d", p=128))
                    wkt[p], wqt[p] = wk, wq

                def emit_kq_chunk(p, idx):
                    if idx == 0:
                        kT[p] = kqp.tile([128, N], f32r, tag="kT", bufs=2,
                                         name=f"kT{p}")
                        qT[p] = kqp.tile([128, HALF], f32r, tag="qT", bufs=2,
                                         name=f"qT{p}")
                    ps = projp.tile([128, 512], f32, tag="proj", bufs=1,
                                    name=f"prj{p}_{idx}")
                    if idx < 4:
                        for ct in range(CT):
                            nc.tensor.matmul(
                                ps[:], wkt[p][:, ct * 128:(ct + 1) * 128],
                                xT[ct][:, idx * 512:(idx + 1) * 512],
                                start=(ct == 0), stop=(ct == CT - 1))
                        nc.vector.tensor_copy(kT[p][:, idx * 512:(idx + 1) * 512],
                                              ps[:])
                    else:
                        q = idx - 4
                        for ct in range(CT):
                            nc.tensor.matmul(
                                ps[:], wqt[p][:, ct * 128:(ct + 1) * 128],
                                xT[ct][:, HALF + q * 512:HALF + (q + 1) * 512],
                                start=(ct == 0), stop=(ct == CT - 1))
                        nc.vector.tensor_copy(qT[p][:, q * 512:(q + 1) * 512],
                                              ps[:])

                def emit_v_bundle(p, b):
                    ps = projp.tile([128, 512], f32, tag="proj", bufs=1,
                                    name=f"vb{p}_{b}")
                    for jq in range(4):
                        jt = 4 * b + jq
                        for ct in range(CT):
                            nc.tensor.matmul(
                                ps[:, jq * 128:(jq + 1) * 128],
                                xT[ct][:, jt * 128:(jt + 1) * 128],
                                wv_t[ct][:, p * 128:(p + 1) * 128],
                                start=(ct == 0), stop=(ct == CT - 1))
                    src = ps.rearrange("p (j h c) -> p j h c", j=4, h=2)
                    dst = v_nat.rearrange("p (j hh c) -> p j hh c", hh=HEADS,
                                          c=VROW)[:, 4 * b:4 * b + 4,
                                                  2 * p:2 * p + 2, 0:DH]
                    nc.vector.tensor_copy(dst, src)

                # slots are laid out in "decades" of 10: an even (1536-wide,
                # 6-slot) group then an odd (1024-wide, 4-slot) group, so the
                # pp double-buffer fits 5 PSUM banks (3 + 2).
                def slot_group(s):
                    d, r = divmod(s, 10)
                    if r < 6:
                        return 2 * d, r * SLOT
                    return 2 * d + 1, (r - 6) * SLOT

                def group_slots(g):
                    d, par = divmod(g, 2)
                    base = d * 10 + (6 if par else 0)
                    n = 4 if par else 6
                    return range(base, min(NSLOT, base + n))

                def emit_dots(s):
                    # one f32r matmul covers slots (s, s+1): 512-wide moving
                    # (f32r matmuls with a 256-wide moving operand hang the HW)
                    pair, cq, jt, h, ihalf = decode(s)
                    if ihalf != 0:
                        return
                    g, col = slot_group(s)
                    if g not in pp_tiles:
                        if g % 2 == 0:
                            pp_tiles[g] = ppp.tile([128, 6 * SLOT], f32,
                                                   tag="ppA", bufs=1,
                                                   name=f"pp{g}")
                        else:
                            pp_tiles[g] = ppp.tile([128, 4 * SLOT], f32,
                                                   tag="ppB", bufs=1,
                                                   name=f"pp{g}")
                    nc.tensor.matmul(
                        pp_tiles[g][:, col:col + 512],
                        kT[pair][h * 64:(h + 1) * 64, jt * 128:(jt + 1) * 128],
                        qT[pair][h * 64:(h + 1) * 64, cq * 512:(cq + 1) * 512],
                        start=True, stop=True)

                def emit_exp(g):
                    n = len(group_slots(g))
                    w = n * SLOT
                    if g % 2 == 0:
                        at = ringp.tile([128, 6 * SLOT], f16, tag="atA", bufs=7,
                                        name=f"at{g}")
                    else:
                        at = ringp.tile([128, 4 * SLOT], f16, tag="atB", bufs=7,
                                        name=f"at{g}")
                    nc.scalar.activation(at[:, 0:w], pp_tiles[g][:, 0:w], EXP,
                                         bias=0.0, scale=SCALE)
                    at_tiles[g] = at
                    del pp_tiles[g]

                def slot_of(pair, cq, jt, h, ihalf):
                    return ((pair * 2 + cq) * 16 + jt) * 4 + h * 2 + ihalf

                def av_mm(s, it):
                    # one AV matmul: at-slice stationary, v (+ones col) moving
                    pair, cq, jt, h, ihalf = decode(s)
                    g, col = slot_group(s)
                    po = po_tiles[(pair, cq)][h]
                    hg = 2 * pair + h
                    nc.tensor.matmul(
                        po[:, it * VROW:(it + 1) * VROW],
                        at_tiles[g][:, col + (it % 2) * 128:
                                    col + (it % 2 + 1) * 128],
                        v_nat[:, jt * HEADS * VROW + hg * VROW:
                              jt * HEADS * VROW + (hg + 1) * VROW],
                        start=(jt == 0), stop=(jt == JT - 1))

                def emit_norm(pair, cq):
                    poh = po_tiles.pop((pair, cq))
                    for h in range(2):
                        for it in range(4):
                            off = it * VROW
                            rs = smallp.tile([128, 1], f32, tag="rs", bufs=8,
                                             name=f"rs{pair}_{cq}_{h}_{it}")
                            nc.vector.reciprocal(rs[:],
                                                 poh[h][:, off + DH:off + DH + 1])
                            itg = cq * 4 + it
                            nc.vector.tensor_scalar(
                                ctx_t[pair][itg][:, h * 64:(h + 1) * 64],
                                poh[h][:, off:off + DH], rs[:], None, op0=MULT)

                def emit_av(g):
                    # streamed it=0 chains for this group's ihalf=0 slots; when
                    # a (pair, cq) chunk's last slot is exp'd, burst the it=1..3
                    # chains (sequential per PSUM bank, interleaved across the
                    # two banks) and normalize.
                    for s in group_slots(g):
                        pair, cq, jt, h, ihalf = decode(s)
                        if (pair, cq) not in po_tiles:
                            poA = pop.tile([128, 512], f32, tag="poA", bufs=1,
                                           name=f"poA{pair}_{cq}")
                            poB = pop.tile([128, 512], f32, tag="poB", bufs=1,
                                           name=f"poB{pair}_{cq}")
                            po_tiles[(pair, cq)] = (poA, poB)
                        if ihalf == 0:
                            av_mm(s, 0)
                        if jt == JT - 1 and h == 1 and ihalf == 1:
                            for it in (1, 2, 3):
                                for jt2 in range(JT):
                                    for h2 in range(2):
                                        av_mm(slot_of(pair, cq, jt2, h2, it // 2),
                                              it)
                            emit_norm(pair, cq)

                # ---- schedule ----
                proj_q = deque()
                projected = set()

                def push_pair(p):
                    if p < PAIRS and p not in projected:
                        projected.add(p)
                        proj_q.append(("w", p))
                        for i in range(6):
                            proj_q.append(("kq", p, i))
                        for b in range(4):
                            proj_q.append(("v", p, b))

                def pop_bundle():
                    if not proj_q:
                        return
                    b = proj_q.popleft()
                    if b[0] == "w":
                        emit_wdma(b[1])
                    elif b[0] == "kq":
                        emit_kq_chunk(b[1], b[2])
                    else:
                        emit_v_bundle(b[1], b[2])

                push_pair(0)
                while proj_q:
                    pop_bundle()
                push_pair(1)

                last_exp = -1
                for s in range(NSLOT):
                    pair = s // SPP
                    if s % SPP == 0:
                        while proj_q and proj_q[0][1] <= pair:
                            pop_bundle()
                        push_pair(pair + 1)
                    emit_dots(s)
                    r = s % 10
                    if r == 5 or r == 9 or s == NSLOT - 1:
                        g, _ = slot_group(s)
                        emit_exp(g)
                        if g > 0:
                            emit_av(g - 1)
                        last_exp = g
                        if g % 2 == 0:
                            pop_bundle()
                emit_av(last_exp)
                while proj_q:
                    pop_bundle()

            # ---- tail: transpose ctx to [d, i] and out-project ----
            with tc.tile_pool(name="tail", bufs=1) as tailp, \
                 tc.tile_pool(name="tpsum", bufs=1, space="PSUM") as tpsum:
                for itg in range(8):
                    ctxT = tailp.tile([128, PAIRS * 128], f16, tag="ctxT", bufs=2,
                                      name=f"ctxT{itg}")
                    for p in range(PAIRS):
                        tp = tpsum.tile([128, 128], f32r, tag="tp", bufs=4,
                                        name=f"tp{itg}_{p}")
                        nc.tensor.transpose(tp[:], ctx_t[p][itg][:], ident[:])
                        nc.vector.tensor_copy(ctxT[:, p * 128:(p + 1) * 128], tp[:])
                    for ep in range(2):
                        yp = tpsum.tile([128, 512], f32, tag="yp", bufs=2,
                                        name=f"yp{itg}_{ep}")
                        for p in range(PAIRS):
                            nc.tensor.matmul(
                                yp[:], ctxT[:, p * 128:(p + 1) * 128],
                                wout_t[p][:, ep * 512:(ep + 1) * 512],
                                start=(p == 0), stop=(p == PAIRS - 1))
                        ysb = tailp.tile([128, 512], f32, tag="ysb", bufs=3,
                                         name=f"ysb{itg}_{ep}")
                        nc.vector.tensor_add(ysb[:], yp[:],
                                             bias[:, ep * 512:(ep + 1) * 512])
                        nc.sync.dma_start(
                            Y[itg * 128:(itg + 1) * 128, ep * 512:(ep + 1) * 512],
                            ysb[:])

    nc.compile()
    return nc


def _get_compiled():
    global _compiled
    if _compiled is None:
        _compiled = _build()
    return _compiled


def make_in_maps(x, w_qkv, w_out, b_out):
    x = np.asarray(x, dtype=np.float32)
    w_qkv = np.asarray(w_qkv, dtype=np.float32)
    w_out = np.asarray(w_out, dtype=np.float32)
    b_out = np.asarray(b_out, dtype=np.float32)

    wq = np.ascontiguousarray(w_qkv[:, 0:DIM].astype(np.float16))
    wk = np.ascontiguousarray(w_qkv[:, DIM:2 * DIM].astype(np.float16))
    wv = np.ascontiguousarray(w_qkv[:, 2 * DIM:3 * DIM].astype(np.float16))
    wo = np.ascontiguousarray(w_out.astype(np.float16))

    in_maps = []
    for c in range(NC):
        b, half = divmod(c, 2)
        other = x[b][(1 - half) * HALF:(2 - half) * HALF]
        mine = x[b][half * HALF:(half + 1) * HALF]
        xb = np.ascontiguousarray(
            np.concatenate([other, mine], axis=0).T.astype(np.float16))
        in_maps.append({"x": xb, "wv": wv, "wk": wk, "wq": wq, "wo": wo,
                        "b_out": b_out})
    return in_maps


def kernel(x, w_qkv, w_out, b_out):
    from concourse.bass_utils import run_bass_kernel_spmd

    nc = _get_compiled()
    in_maps = make_in_maps(x, w_qkv, w_out, b_out)
    res = run_bass_kernel_spmd(nc, in_maps, core_ids=list(range(NC)))

    out = np.empty((B, N, DIM), dtype=np.float32)
    for c in range(NC):
        b, half = divmod(c, 2)
        out[b, half * HALF:(half + 1) * HALF] = res.results[c]["y"]
    return out
